# revision 2
# baseline (speedup 1.0000x reference)
"""MoE layer (8 experts, top-2 routing, SwiGLU) on 8 Trainium2 NeuronCores.

Strategy (expert-parallel, bf16):
  - Host: run the (tiny) router matmul + softmax + top-2 in numpy, sort the
    (token, slot) pairs by expert id, and build per-expert gathered token
    batches padded to a common capacity. The per-token gate is folded into a
    second, pre-scaled copy of x (the w3 operand), so the device never
    multiplies by the gate. All operands are cast to bf16 on the host.
  - Device (SPMD, core e == expert e):
        y = (silu(x @ w1e.T) * ((g*x) @ w3e.T)) @ w2e.T
    in a feature-major layout (no on-chip transposes). bf16 operands give
    1 cycle/row on the PE with fast weight load (FWL), so LDWEIGHTS hides
    fully under the matmuls; accumulation stays fp32 in PSUM.
  - Host: un-permute and add the two expert contributions per token.

B, T, C, E, H = 8, 2048, 256, 8, 682; N = B*T = 16384 tokens, top-2.
"""

import os

import ml_dtypes
import numpy as np

import concourse.bass as bass
import concourse.tile as tile
from concourse import bacc, mybir
from concourse.bass_utils import run_bass_kernel_spmd

E = 8
TOP_K = 2
C = 256
H = 682
NTILE = 512  # moving-dim tile (fp32 PSUM bank width)
H_CHUNKS = [(0, 128), (128, 128), (256, 128), (384, 128), (512, 128), (640, 42)]
C_CHUNKS = [(0, 128), (128, 128)]
N_WARMUP_MM = 10  # bf16 dummy matmuls keeping the PE HAM warm across the
# startup DMA window (HAM un-throttles after ~3.4us of sustained PE work)

BF16 = ml_dtypes.bfloat16

_PROGRAM_CACHE: dict[int, object] = {}


def _route(flat: np.ndarray, router_w: np.ndarray):
    """Replicates the reference router: softmax over experts, top-2, renorm."""
    logits = flat @ router_w.T  # [N, E]
    logits -= logits.max(axis=-1, keepdims=True)
    probs = np.exp(logits)
    probs /= probs.sum(axis=-1, keepdims=True)

    n = flat.shape[0]
    ar = np.arange(n)
    i0 = probs.argmax(axis=-1)
    p0 = probs[ar, i0]
    masked = probs.copy()
    masked[ar, i0] = -np.inf
    i1 = masked.argmax(axis=-1)
    p1 = probs[ar, i1]
    denom = p0 + p1 + 1e-9
    return i0, i1, (p0 / denom).astype(np.float32), (p1 / denom).astype(np.float32)


def _n_tiles(cap: int):
    """n-tile (offset, size) list: 512-wide tiles plus one >=128 tail."""
    tiles, off = [], 0
    while cap - off > NTILE:
        tiles.append((off, NTILE))
        off += NTILE
    tiles.append((off, cap - off))
    return tiles


def _build_program(cap: int):
    f32 = mybir.dt.float32
    bf16 = mybir.dt.bfloat16
    ntiles = _n_tiles(cap)
    nt = len(ntiles)

    nc = bacc.Bacc(
        "TRN2",
        target_bir_lowering=False,
        debug=False,
        enable_asserts=False,
        num_devices=E,
    )
    x1T_d = nc.dram_tensor("x1T", [C, cap], bf16, kind="ExternalInput").ap()
    x3T_d = nc.dram_tensor("x3T", [C, cap], bf16, kind="ExternalInput").ap()
    w1T_d = nc.dram_tensor("w1T", [C, H], bf16, kind="ExternalInput").ap()
    w3T_d = nc.dram_tensor("w3T", [C, H], bf16, kind="ExternalInput").ap()
    w2T_d = nc.dram_tensor("w2T", [H, C], bf16, kind="ExternalInput").ap()
    yT_d = nc.dram_tensor("yT", [C, cap], bf16, kind="ExternalOutput").ap()

    with tile.TileContext(nc) as tc:
        with (
            tc.tile_pool(name="consts", bufs=1) as consts,
            tc.tile_pool(name="xin", bufs=3) as xin,
            tc.tile_pool(name="hbuf", bufs=3) as hbuf,
            tc.tile_pool(name="act", bufs=4) as actp,
            tc.tile_pool(name="yout", bufs=4) as yout,
            tc.tile_pool(name="ps_h", bufs=2, space="PSUM") as ps_h,
            tc.tile_pool(name="ps_y", bufs=3, space="PSUM") as ps_y,
            tc.tile_pool(name="ps_w", bufs=1, space="PSUM") as ps_w,
        ):
            # PE warm-up: dummy matmuls on zeroed SBUF keep the HAM busy
            # (and warm) while the first input DMAs are in flight.
            wz_l = consts.tile([128, 512], bf16, tag="wz_l")
            nc.vector.memset(wz_l[:].bitcast(mybir.dt.uint16), 0)
            for _ in range(N_WARMUP_MM):
                wp = ps_w.tile([128, 512], f32, tag="warm")
                nc.tensor.matmul(
                    wp[:], wz_l[:, :128], wz_l[:], start=True, stop=True
                )

            x_tiles: dict[int, list] = {}

            def load_x(j):
                no, nsz = ntiles[j]
                ts = []
                for ci, (co, _) in enumerate(C_CHUNKS):
                    x1t = xin.tile([128, nsz], bf16, tag=f"x1{ci}")
                    nc.sync.dma_start(
                        out=x1t[:], in_=x1T_d[co : co + 128, no : no + nsz]
                    )
                    x3t = xin.tile([128, nsz], bf16, tag=f"x3{ci}")
                    nc.sync.dma_start(
                        out=x3t[:], in_=x3T_d[co : co + 128, no : no + nsz]
                    )
                    ts.append((x1t, x3t))
                x_tiles[j] = ts

            # Critical-path first: the opening matmul needs x(j0,c0) + the
            # first w1 chunk. Weights load per-[128,128] tile so the first
            # matmul only waits on 32KB of weight DMA. w2 rides the gpsimd
            # queue and is emitted after the first h-phase.
            w1_sb: dict[tuple, object] = {}
            w3_sb: dict[tuple, object] = {}
            w2_sb: dict[tuple, object] = {}
            load_x(0)
            for hi, (ho, hs) in enumerate(H_CHUNKS):
                for ci, (co, _) in enumerate(C_CHUNKS):
                    t1 = consts.tile([128, hs], bf16, tag=f"w1c{ci}h{hi}")
                    nc.sync.dma_start(
                        out=t1[:], in_=w1T_d[co : co + 128, ho : ho + hs]
                    )
                    w1_sb[(ci, hi)] = t1
                for ci, (co, _) in enumerate(C_CHUNKS):
                    t3 = consts.tile([128, hs], bf16, tag=f"w3c{ci}h{hi}")
                    nc.sync.dma_start(
                        out=t3[:], in_=w3T_d[co : co + 128, ho : ho + hs]
                    )
                    w3_sb[(ci, hi)] = t3
            load_x(1)

            def emit_h_phase(j):
                """h = silu(x@w1T) * (gx@w3T) for n-tile j; returns SBUF tiles."""
                no, nsz = ntiles[j]
                x_sb = x_tiles.pop(j)
                h_tiles = []
                for hi, (ho, hs) in enumerate(H_CHUNKS):
                    h1p = ps_h.tile([hs, nsz], f32, tag="h1")
                    h3p = ps_h.tile([hs, nsz], f32, tag="h3")
                    for ci in range(len(C_CHUNKS)):
                        first = ci == 0
                        last = ci == len(C_CHUNKS) - 1
                        nc.tensor.matmul(
                            h1p[:],
                            w1_sb[(ci, hi)][:],
                            x_sb[ci][0][:],
                            start=first,
                            stop=last,
                        )
                        nc.tensor.matmul(
                            h3p[:],
                            w3_sb[(ci, hi)][:],
                            x_sb[ci][1][:],
                            start=first,
                            stop=last,
                        )
                    a_sb = actp.tile([hs, nsz], f32, tag="a")
                    nc.scalar.activation(
                        a_sb[:], h1p[:], mybir.ActivationFunctionType.Silu
                    )
                    h_sb = hbuf.tile([hs, nsz], bf16, tag=f"h{hi}")
                    nc.vector.tensor_mul(h_sb[:], a_sb[:], h3p[:])
                    h_tiles.append(h_sb)
                return (h_tiles,)

            def emit_y_phase(j, h_tiles):
                no, nsz = ntiles[j]
                for ci, (co, _) in enumerate(C_CHUNKS):
                    yp = ps_y.tile([128, nsz], f32, tag="y")
                    for hi in range(len(H_CHUNKS)):
                        nc.tensor.matmul(
                            yp[:],
                            w2_sb[(hi, ci)][:],
                            h_tiles[hi][:],
                            start=hi == 0,
                            stop=hi == len(H_CHUNKS) - 1,
                        )
                    y_sb = yout.tile([128, nsz], bf16, tag="yo")
                    nc.scalar.activation(
                        y_sb[:], yp[:], mybir.ActivationFunctionType.Copy
                    )
                    nc.sync.dma_start(
                        out=yT_d[co : co + 128, no : no + nsz], in_=y_sb[:]
                    )

            # Software pipeline: y-phase of tile j is emitted after the
            # h-phase of tile j+1, so the PE never waits on the silu->mul
            # chain at the h->y boundary.
            pending = None
            for j in range(nt):
                if j + 2 < nt:
                    load_x(j + 2)
                hj = emit_h_phase(j)
                if j == 0:
                    for hi, (ho, hs) in enumerate(H_CHUNKS):
                        for ci, (co, _) in enumerate(C_CHUNKS):
                            t2 = consts.tile([hs, 128], bf16, tag=f"w2h{hi}c{ci}")
                            nc.gpsimd.dma_start(
                                out=t2[:], in_=w2T_d[ho : ho + hs, co : co + 128]
                            )
                            w2_sb[(hi, ci)] = t2
                if pending is not None:
                    emit_y_phase(*pending)
                pending = (j, *hj)
            emit_y_phase(*pending)

    nc.compile()
    return nc


def _get_program(cap: int):
    if cap not in _PROGRAM_CACHE:
        _PROGRAM_CACHE[cap] = _build_program(cap)
    return _PROGRAM_CACHE[cap]


def kernel(x, router_w, w1, w2, w3, _trace=False):
    B, T, _ = x.shape
    n = B * T
    flat = np.ascontiguousarray(x.reshape(n, C), dtype=np.float32)
    i0, i1, g0, g1 = _route(flat, np.asarray(router_w, dtype=np.float32))

    # Dispatch: for each expert, the token rows routed to it (slot0 then slot1).
    pos = np.empty((2, n), dtype=np.int64)  # row of each (slot, token) in Y
    in_maps = []
    counts = [
        (np.nonzero(i0 == e)[0], np.nonzero(i1 == e)[0]) for e in range(E)
    ]
    cap = max(len(s0) + len(s1) for s0, s1 in counts)
    cap = max(((cap + 127) // 128) * 128, 256)

    w1 = np.asarray(w1, dtype=np.float32)
    w2 = np.asarray(w2, dtype=np.float32)
    w3 = np.asarray(w3, dtype=np.float32)
    for e in range(E):
        s0, s1 = counts[e]
        cnt = len(s0) + len(s1)
        base = e * cap
        pos[0, s0] = base + np.arange(len(s0))
        pos[1, s1] = base + len(s0) + np.arange(len(s1))

        xg = np.zeros((C, cap), dtype=np.float32)
        xg[:, : len(s0)] = flat[s0].T
        xg[:, len(s0) : cnt] = flat[s1].T
        x1T = xg.astype(BF16)
        # gate folded into the w3 operand: y = silu(x@w1T) * ((g*x)@w3T) @ w2T
        xg[:, : len(s0)] *= g0[s0]
        xg[:, len(s0) : cnt] *= g1[s1]
        x3T = xg.astype(BF16)
        in_maps.append(
            {
                "x1T": x1T,
                "x3T": x3T,
                "w1T": np.ascontiguousarray(w1[e].T).astype(BF16),
                "w3T": np.ascontiguousarray(w3[e].T).astype(BF16),
                "w2T": np.ascontiguousarray(w2[e].T).astype(BF16),
            }
        )

    nc = _get_program(cap)
    if _trace:
        res = run_bass_kernel_spmd(nc, in_maps, list(range(E)), trace=True)
    else:
        # The NTFF trace path needs an antenv.axon_hooks shim this module
        # doesn't install; make sure an ambient BASS_TRACE can't enable it.
        prev = os.environ.get("BASS_NEVER_TRACE")
        os.environ["BASS_NEVER_TRACE"] = "1"
        try:
            res = run_bass_kernel_spmd(nc, in_maps, list(range(E)), trace=False)
        finally:
            if prev is None:
                os.environ.pop("BASS_NEVER_TRACE", None)
            else:
                os.environ["BASS_NEVER_TRACE"] = prev

    Y = np.empty((E * cap, C), dtype=np.float32)
    for e in range(E):
        Y[e * cap : (e + 1) * cap] = res.results[e]["yT"].T.astype(np.float32)
    out = Y[pos[0]] + Y[pos[1]]
    if _trace:
        kernel.last_results = res
    return out.reshape(B, T, C)


# revision 3
# speedup vs baseline: 1.1309x; 1.1309x over previous
"""MoE layer (8 experts, top-2 routing, SwiGLU) on 8 Trainium2 NeuronCores.

Strategy (expert-parallel):
  - Host: run the (tiny) router matmul + softmax + top-2 in numpy, sort the
    (token, slot) pairs by expert id, and build per-expert gathered token
    batches padded to a common capacity.
  - Device (SPMD, core e == expert e): y = (silu(x @ w1e.T) * (x @ w3e.T)) @ w2e.T
    scaled by the per-token gate, all in a feature-major layout so no
    on-chip transposes are needed. Matmuls run in float32r (full fp32
    storage, 1 cycle/row on the PE for moving dim >= 256) — measured
    faster per-instruction than bf16 on this silicon (226 vs 258 ns for
    a 512-wide matmul; bf16 appears to trip the P0 power downclock).
  - Host: un-permute and add the two expert contributions per token.

B, T, C, E, H = 8, 2048, 256, 8, 682; N = B*T = 16384 tokens, top-2.
"""

import os

import ml_dtypes
import numpy as np

import concourse.bass as bass
import concourse.tile as tile
from concourse import bacc, mybir
from concourse.bass_utils import run_bass_kernel_spmd

E = 8
TOP_K = 2
C = 256
H = 682
HP = 768  # H zero-padded to a multiple of 128: partial-row (42-wide)
# fp32 LDWEIGHTS stalls the PE for ~2 matmuls; padding is free PE-wise
# (matmul cost is moving-dim cycles only) and numerically exact (zero
# weights -> silu(0)*0 = 0 contribution).
NTILE = 512  # moving-dim tile (fp32 PSUM bank width)
H_CHUNKS = [(i * 128, 128) for i in range(HP // 128)]
C_CHUNKS = [(i * 128, 128) for i in range(C // 128)]
N_WARMUP_MM = 5  # fp32 dummy matmuls (2 HW mms each, ~450ns cold) covering
# the startup DMA window to hold the PE HAM warm without overshooting the
# arrival of the first x tile (first n-tile is small for the same reason)

BF16 = ml_dtypes.bfloat16

_PROGRAM_CACHE: dict[int, object] = {}


def _route(flat: np.ndarray, router_w: np.ndarray):
    """Replicates the reference router: softmax over experts, top-2, renorm."""
    logits = flat @ router_w.T  # [N, E]
    logits -= logits.max(axis=-1, keepdims=True)
    probs = np.exp(logits)
    probs /= probs.sum(axis=-1, keepdims=True)

    n = flat.shape[0]
    ar = np.arange(n)
    i0 = probs.argmax(axis=-1)
    p0 = probs[ar, i0]
    masked = probs.copy()
    masked[ar, i0] = -np.inf
    i1 = masked.argmax(axis=-1)
    p1 = probs[ar, i1]
    denom = p0 + p1 + 1e-9
    return i0, i1, (p0 / denom).astype(np.float32), (p1 / denom).astype(np.float32)


def _n_tiles(cap: int):
    """n-tile (offset, size) list: a small (256..512) head tile so the first
    matmul only waits on a short DMA, then 512-wide tiles (all >=256 wide,
    the f32r fast-path minimum)."""
    rem = cap % NTILE
    if rem == 0:
        sizes = [NTILE] * (cap // NTILE)
    elif rem >= 256:
        sizes = [rem] + [NTILE] * (cap // NTILE)
    else:
        a = (rem + NTILE) // 2
        a = ((a + 15) // 16) * 16
        sizes = [a, rem + NTILE - a] + [NTILE] * (cap // NTILE - 1)
    tiles, off = [], 0
    for s in sizes:
        tiles.append((off, s))
        off += s
    assert off == cap
    return tiles


def _build_program(cap: int):
    f32 = mybir.dt.float32
    f32r = mybir.dt.float32r
    bf16 = mybir.dt.bfloat16
    ntiles = _n_tiles(cap)
    nt = len(ntiles)

    nc = bacc.Bacc(
        "TRN2",
        target_bir_lowering=False,
        debug=False,
        enable_asserts=False,
        num_devices=E,
    )
    xT_d = nc.dram_tensor("xT", [C, cap], f32r, kind="ExternalInput").ap()
    g_d = nc.dram_tensor("g", [1, cap], f32r, kind="ExternalInput").ap()
    w1T_d = nc.dram_tensor("w1T", [C, H], f32r, kind="ExternalInput").ap()
    w3T_d = nc.dram_tensor("w3T", [C, H], f32r, kind="ExternalInput").ap()
    w2T_d = nc.dram_tensor("w2T", [H, C], f32r, kind="ExternalInput").ap()
    yT_d = nc.dram_tensor("yT", [C, cap], bf16, kind="ExternalOutput").ap()

    with tile.TileContext(nc) as tc:
        with (
            tc.tile_pool(name="consts", bufs=1) as consts,
            tc.tile_pool(name="xin", bufs=3) as xin,
            tc.tile_pool(name="gbp", bufs=3) as gbpool,
            tc.tile_pool(name="hbuf", bufs=3) as hbuf,
            tc.tile_pool(name="act", bufs=4) as actp,
            tc.tile_pool(name="yout", bufs=4) as yout,
            tc.tile_pool(name="ps_h", bufs=2, space="PSUM") as ps_h,
            tc.tile_pool(name="ps_y", bufs=3, space="PSUM") as ps_y,
            tc.tile_pool(name="ps_w", bufs=1, space="PSUM") as ps_w,
        ):
            # PE warm-up: dummy matmuls on zeroed SBUF keep the HAM busy
            # (and warm) while the first input DMAs are in flight.
            wz_l = consts.tile([128, 128], f32, tag="wz_l")
            nc.vector.memset(wz_l[:], 0.0)
            for _ in range(N_WARMUP_MM):
                wp = ps_w.tile([128, 128], f32, tag="warm")
                nc.tensor.matmul(wp[:], wz_l[:], wz_l[:], start=True, stop=True)

            x_tiles: dict[int, list] = {}

            def load_x(j):
                no, nsz = ntiles[j]
                ts = []
                for ci, (co, _) in enumerate(C_CHUNKS):
                    xt = xin.tile([128, nsz], f32r, tag=f"x{ci}")
                    nc.sync.dma_start(
                        out=xt[:], in_=xT_d[co : co + 128, no : no + nsz]
                    )
                    ts.append(xt)
                x_tiles[j] = ts

            # Critical-path first: the opening matmul needs x(j0,c0) + the
            # first w1 chunk. Weights land in per-[128,128] DMA pieces so
            # the first matmuls wait on 64KB, not on a 350KB block. w2 +
            # gate broadcasts ride the gpsimd queue after the first h-phase.
            w1_sb, w3_sb, w2_sb = [], [], []
            load_x(0)
            for ci, (co, _) in enumerate(C_CHUNKS):
                t1 = consts.tile([128, HP], f32r, tag=f"w1c{co}")
                w1_sb.append(t1)
                t3 = consts.tile([128, HP], f32r, tag=f"w3c{co}")
                w3_sb.append(t3)
            for hi, (ho, hs) in enumerate(H_CHUNKS):
                he = min(ho + hs, H)
                for w_sb, w_d in ((w1_sb, w1T_d), (w3_sb, w3T_d)):
                    for ci, (co, _) in enumerate(C_CHUNKS):
                        if he > ho:
                            nc.sync.dma_start(
                                out=w_sb[ci][:, ho:he], in_=w_d[co : co + 128, ho:he]
                            )
                        if he < ho + hs:
                            nc.vector.memset(
                                w_sb[ci][:, max(he, ho) : ho + hs].bitcast(f32), 0.0
                            )
                if hi == 0:
                    load_x(1)

            def emit_h_phase(j):
                """h = silu(x@w1T) * (x@w3T) for n-tile j; returns SBUF tiles."""
                no, nsz = ntiles[j]
                x_sb = x_tiles.pop(j)
                h_tiles = []
                for hi, (ho, hs) in enumerate(H_CHUNKS):
                    h1p = ps_h.tile([hs, nsz], f32, tag="h1")
                    h3p = ps_h.tile([hs, nsz], f32, tag="h3")
                    for ci in range(len(C_CHUNKS)):
                        first = ci == 0
                        last = ci == len(C_CHUNKS) - 1
                        nc.tensor.matmul(
                            h1p[:],
                            w1_sb[ci][:, ho : ho + hs],
                            x_sb[ci][:],
                            start=first,
                            stop=last,
                        )
                        nc.tensor.matmul(
                            h3p[:],
                            w3_sb[ci][:, ho : ho + hs],
                            x_sb[ci][:],
                            start=first,
                            stop=last,
                        )
                    a_sb = actp.tile([hs, nsz], f32r, tag="a")
                    nc.scalar.activation(
                        a_sb[:], h1p[:], mybir.ActivationFunctionType.Silu
                    )
                    h_sb = hbuf.tile([hs, nsz], f32r, tag=f"h{hi}")
                    nc.vector.tensor_mul(h_sb[:], a_sb[:], h3p[:])
                    h_tiles.append(h_sb)
                # gate row broadcast to 128 partitions via stride-0 DMA
                gb_sb = gbpool.tile([128, nsz], f32, tag="gb")
                g_slice = g_d[0:1, no : no + nsz]
                g_bcast = bass.AP(
                    tensor=g_slice.tensor,
                    offset=g_slice.offset,
                    ap=[[0, 128], list(g_slice.ap[-1])],
                )
                nc.gpsimd.dma_start(out=gb_sb[:], in_=g_bcast)
                return h_tiles, gb_sb

            def emit_y_phase(j, h_tiles, gb_sb):
                no, nsz = ntiles[j]
                for ci, (co, _) in enumerate(C_CHUNKS):
                    yp = ps_y.tile([128, nsz], f32, tag="y")
                    for hi, (ho, hs) in enumerate(H_CHUNKS):
                        nc.tensor.matmul(
                            yp[:],
                            w2_sb[hi][:, co : co + 128],
                            h_tiles[hi][:],
                            start=hi == 0,
                            stop=hi == len(H_CHUNKS) - 1,
                        )
                    y_sb = yout.tile([128, nsz], bf16, tag="yo")
                    nc.vector.tensor_mul(y_sb[:], yp[:], gb_sb[:])
                    nc.sync.dma_start(
                        out=yT_d[co : co + 128, no : no + nsz], in_=y_sb[:]
                    )

            # Software pipeline: y-phase of tile j is emitted after the
            # h-phase of tile j+1, so the PE never waits on the silu->mul
            # chain at the h->y boundary.
            pending = None
            for j in range(nt):
                if j + 2 < nt:
                    load_x(j + 2)
                hj = emit_h_phase(j)
                if j == 0:
                    for ho, hs in H_CHUNKS:
                        t2 = consts.tile([hs, C], f32r, tag=f"w2h{ho}")
                        real = min(H - ho, hs)
                        if real < hs:
                            nc.vector.memset(t2.bitcast(f32), 0.0)
                        nc.gpsimd.dma_start(out=t2[:real, :], in_=w2T_d[ho : ho + real, :])
                        w2_sb.append(t2)
                if pending is not None:
                    emit_y_phase(*pending)
                pending = (j, *hj)
            emit_y_phase(*pending)

    nc.compile()
    return nc


def _get_program(cap: int):
    if cap not in _PROGRAM_CACHE:
        _PROGRAM_CACHE[cap] = _build_program(cap)
    return _PROGRAM_CACHE[cap]


def kernel(x, router_w, w1, w2, w3, _trace=False):
    B, T, _ = x.shape
    n = B * T
    flat = np.ascontiguousarray(x.reshape(n, C), dtype=np.float32)
    i0, i1, g0, g1 = _route(flat, np.asarray(router_w, dtype=np.float32))

    # Dispatch: for each expert, the token rows routed to it (slot0 then slot1).
    pos = np.empty((2, n), dtype=np.int64)  # row of each (slot, token) in Y
    in_maps = []
    counts = [
        (np.nonzero(i0 == e)[0], np.nonzero(i1 == e)[0]) for e in range(E)
    ]
    cap = max(len(s0) + len(s1) for s0, s1 in counts)
    cap = max(((cap + 127) // 128) * 128, 256)

    w1 = np.asarray(w1, dtype=np.float32)
    w2 = np.asarray(w2, dtype=np.float32)
    w3 = np.asarray(w3, dtype=np.float32)
    for e in range(E):
        s0, s1 = counts[e]
        cnt = len(s0) + len(s1)
        base = e * cap
        pos[0, s0] = base + np.arange(len(s0))
        pos[1, s1] = base + len(s0) + np.arange(len(s1))

        xT = np.zeros((C, cap), dtype=np.float32)
        xT[:, : len(s0)] = flat[s0].T
        xT[:, len(s0) : cnt] = flat[s1].T
        g = np.zeros((1, cap), dtype=np.float32)
        g[0, : len(s0)] = g0[s0]
        g[0, len(s0) : cnt] = g1[s1]
        in_maps.append(
            {
                "xT": xT,
                "g": g,
                "w1T": np.ascontiguousarray(w1[e].T),
                "w3T": np.ascontiguousarray(w3[e].T),
                "w2T": np.ascontiguousarray(w2[e].T),
            }
        )

    nc = _get_program(cap)
    if _trace:
        res = run_bass_kernel_spmd(nc, in_maps, list(range(E)), trace=True)
    else:
        # The NTFF trace path needs an antenv.axon_hooks shim this module
        # doesn't install; make sure an ambient BASS_TRACE can't enable it.
        prev = os.environ.get("BASS_NEVER_TRACE")
        os.environ["BASS_NEVER_TRACE"] = "1"
        try:
            res = run_bass_kernel_spmd(nc, in_maps, list(range(E)), trace=False)
        finally:
            if prev is None:
                os.environ.pop("BASS_NEVER_TRACE", None)
            else:
                os.environ["BASS_NEVER_TRACE"] = prev

    Y = np.empty((E * cap, C), dtype=np.float32)
    for e in range(E):
        Y[e * cap : (e + 1) * cap] = res.results[e]["yT"].T.astype(np.float32)
    out = Y[pos[0]] + Y[pos[1]]
    if _trace:
        kernel.last_results = res
    return out.reshape(B, T, C)


# revision 6
# speedup vs baseline: 1.1557x; 1.0220x over previous
"""MoE layer (8 experts, top-2 routing, SwiGLU) on 8 Trainium2 NeuronCores.

Strategy (expert-parallel):
  - Host: run the (tiny) router matmul + softmax + top-2 in numpy, sort the
    (token, slot) pairs by expert id, and build per-expert gathered token
    batches padded to a common capacity.
  - Device (SPMD, core e == expert e): y = (silu(x @ w1e.T) * (x @ w3e.T)) @ w2e.T
    scaled by the per-token gate, all in a feature-major layout so no
    on-chip transposes are needed. Matmuls run in float32r (full fp32
    storage, 1 cycle/row on the PE for moving dim >= 256) — measured
    faster per-instruction than bf16 on this silicon (226 vs 258 ns for
    a 512-wide matmul; bf16 appears to trip the P0 power downclock).
  - Host: un-permute and add the two expert contributions per token.

B, T, C, E, H = 8, 2048, 256, 8, 682; N = B*T = 16384 tokens, top-2.
"""

import os

import ml_dtypes
import numpy as np

import concourse.bass as bass
import concourse.tile as tile
from concourse import bacc, mybir
from concourse.bass_utils import run_bass_kernel_spmd

E = 8
TOP_K = 2
C = 256
H = 682
HP = 768  # H zero-padded to a multiple of 128: partial-row (42-wide)
# fp32 LDWEIGHTS stalls the PE for ~2 matmuls; padding is free PE-wise
# (matmul cost is moving-dim cycles only) and numerically exact (zero
# weights -> silu(0)*0 = 0 contribution).
NTILE = 512  # moving-dim tile (fp32 PSUM bank width)
H_CHUNKS = [(i * 128, 128) for i in range(HP // 128)]
C_CHUNKS = [(i * 128, 128) for i in range(C // 128)]
N_WARMUP_MM = 5  # fp32 dummy matmuls (2 HW mms each, ~450ns cold) covering
# the startup DMA window to hold the PE HAM warm without overshooting the
# arrival of the first x tile (first n-tile is small for the same reason)

BF16 = ml_dtypes.bfloat16

_PROGRAM_CACHE: dict[int, object] = {}


def _route(flat: np.ndarray, router_w: np.ndarray):
    """Replicates the reference router: softmax over experts, top-2, renorm."""
    logits = flat @ router_w.T  # [N, E]
    logits -= logits.max(axis=-1, keepdims=True)
    probs = np.exp(logits)
    probs /= probs.sum(axis=-1, keepdims=True)

    n = flat.shape[0]
    ar = np.arange(n)
    i0 = probs.argmax(axis=-1)
    p0 = probs[ar, i0]
    masked = probs.copy()
    masked[ar, i0] = -np.inf
    i1 = masked.argmax(axis=-1)
    p1 = probs[ar, i1]
    denom = p0 + p1 + 1e-9
    return i0, i1, (p0 / denom).astype(np.float32), (p1 / denom).astype(np.float32)


def _n_tiles(cap: int):
    """n-tile (offset, size) list: a small (256..512) head tile so the first
    matmul only waits on a short DMA, then 512-wide tiles (all >=256 wide,
    the f32r fast-path minimum)."""
    rem = cap % NTILE
    if rem == 0:
        sizes = [NTILE] * (cap // NTILE)
    elif rem >= 256:
        sizes = [rem] + [NTILE] * (cap // NTILE)
    else:
        a = (rem + NTILE) // 2
        a = ((a + 15) // 16) * 16
        sizes = [a, rem + NTILE - a] + [NTILE] * (cap // NTILE - 1)
    tiles, off = [], 0
    for s in sizes:
        tiles.append((off, s))
        off += s
    assert off == cap
    return tiles


def _build_program(cap: int):
    f32 = mybir.dt.float32
    f32r = mybir.dt.float32r
    bf16 = mybir.dt.bfloat16
    ntiles = _n_tiles(cap)
    nt = len(ntiles)

    nc = bacc.Bacc(
        "TRN2",
        target_bir_lowering=False,
        debug=False,
        enable_asserts=False,
        num_devices=E,
    )
    xT_d = nc.dram_tensor("xT", [C, cap], f32r, kind="ExternalInput").ap()
    g_d = nc.dram_tensor("g", [1, cap], f32r, kind="ExternalInput").ap()
    w1T_d = nc.dram_tensor("w1T", [C, H], f32r, kind="ExternalInput").ap()
    w3T_d = nc.dram_tensor("w3T", [C, H], f32r, kind="ExternalInput").ap()
    w2T_d = nc.dram_tensor("w2T", [H, C], f32r, kind="ExternalInput").ap()
    yT_d = nc.dram_tensor("yT", [C, cap], bf16, kind="ExternalOutput").ap()

    with tile.TileContext(nc) as tc:
        with (
            tc.tile_pool(name="consts", bufs=1) as consts,
            tc.tile_pool(name="xin", bufs=3) as xin,
            tc.tile_pool(name="gbp", bufs=3) as gbpool,
            tc.tile_pool(name="hbuf", bufs=3) as hbuf,
            tc.tile_pool(name="act", bufs=4) as actp,
            tc.tile_pool(name="yout", bufs=4) as yout,
            tc.tile_pool(name="ps_h", bufs=2, space="PSUM") as ps_h,
            tc.tile_pool(name="ps_y", bufs=3, space="PSUM") as ps_y,
            tc.tile_pool(name="ps_w", bufs=1, space="PSUM") as ps_w,
        ):
            # PE warm-up: dummy matmuls on zeroed SBUF keep the HAM busy
            # (and warm) while the first input DMAs are in flight.
            wz_l = consts.tile([128, 64], f32, tag="wz_l")
            nc.vector.memset(wz_l[:], 0.0)
            for _ in range(N_WARMUP_MM):
                wp = ps_w.tile([64, 64], f32, tag="warm")
                nc.tensor.matmul(wp[:], wz_l[:, :64], wz_l[:], start=True, stop=True)

            x_tiles: dict[int, list] = {}

            def load_x(j):
                no, nsz = ntiles[j]
                ts = []
                for ci, (co, _) in enumerate(C_CHUNKS):
                    xt = xin.tile([128, nsz], f32r, tag=f"x{ci}")
                    nc.sync.dma_start(
                        out=xt[:], in_=xT_d[co : co + 128, no : no + nsz]
                    )
                    ts.append(xt)
                x_tiles[j] = ts

            # Critical-path first: the opening matmul needs x(j0,c0) + the
            # first w1 chunk. Weights land in per-[128,128] DMA pieces so
            # the first matmuls wait on 64KB, not on a 350KB block. w2 +
            # gate broadcasts ride the gpsimd queue after the first h-phase.
            w1_sb, w3_sb, w2_sb = {}, {}, {}
            load_x(0)
            for hi, (ho, hs) in enumerate(H_CHUNKS):
                he = min(ho + hs, H)
                for w_sb, w_d, wn in ((w1_sb, w1T_d, 1), (w3_sb, w3T_d, 3)):
                    for ci, (co, _) in enumerate(C_CHUNKS):
                        t = consts.tile([128, hs], f32r, tag=f"w{wn}c{ci}h{hi}")
                        if he < ho + hs:
                            nc.vector.memset(t[:, he - ho :].bitcast(f32), 0.0)
                        nc.sync.dma_start(
                            out=t[:, : he - ho], in_=w_d[co : co + 128, ho:he]
                        )
                        w_sb[(ci, hi)] = t
                if hi == 0:
                    load_x(1)

            def emit_h_phase(j):
                """h = silu(x@w1T) * (x@w3T) for n-tile j; returns SBUF tiles."""
                no, nsz = ntiles[j]
                x_sb = x_tiles.pop(j)
                h_tiles = []
                for hi, (ho, hs) in enumerate(H_CHUNKS):
                    h1p = ps_h.tile([hs, nsz], f32, tag="h1")
                    h3p = ps_h.tile([hs, nsz], f32, tag="h3")
                    for ci in range(len(C_CHUNKS)):
                        first = ci == 0
                        last = ci == len(C_CHUNKS) - 1
                        nc.tensor.matmul(
                            h1p[:],
                            w1_sb[(ci, hi)][:],
                            x_sb[ci][:],
                            start=first,
                            stop=last,
                        )
                        nc.tensor.matmul(
                            h3p[:],
                            w3_sb[(ci, hi)][:],
                            x_sb[ci][:],
                            start=first,
                            stop=last,
                        )
                    a_sb = actp.tile([hs, nsz], f32r, tag="a")
                    nc.scalar.activation(
                        a_sb[:], h1p[:], mybir.ActivationFunctionType.Silu
                    )
                    h_sb = hbuf.tile([hs, nsz], f32r, tag=f"h{hi}")
                    nc.vector.tensor_mul(h_sb[:], a_sb[:], h3p[:])
                    h_tiles.append(h_sb)
                # gate row broadcast to 128 partitions via stride-0 DMA
                gb_sb = gbpool.tile([128, nsz], f32, tag="gb")
                g_slice = g_d[0:1, no : no + nsz]
                g_bcast = bass.AP(
                    tensor=g_slice.tensor,
                    offset=g_slice.offset,
                    ap=[[0, 128], list(g_slice.ap[-1])],
                )
                nc.gpsimd.dma_start(out=gb_sb[:], in_=g_bcast)
                return h_tiles, gb_sb

            def emit_y_phase(j, h_tiles, gb_sb):
                no, nsz = ntiles[j]
                for ci, (co, _) in enumerate(C_CHUNKS):
                    yp = ps_y.tile([128, nsz], f32, tag="y")
                    for hi, (ho, hs) in enumerate(H_CHUNKS):
                        nc.tensor.matmul(
                            yp[:],
                            w2_sb[(hi, ci)][:],
                            h_tiles[hi][:],
                            start=hi == 0,
                            stop=hi == len(H_CHUNKS) - 1,
                        )
                    y_sb = yout.tile([128, nsz], bf16, tag="yo")
                    nc.vector.tensor_mul(y_sb[:], yp[:], gb_sb[:])
                    nc.sync.dma_start(
                        out=yT_d[co : co + 128, no : no + nsz], in_=y_sb[:]
                    )

            # Software pipeline: y-phase of tile j is emitted after the
            # h-phase of tile j+1, so the PE never waits on the silu->mul
            # chain at the h->y boundary.
            pending = None
            for j in range(nt):
                if j + 2 < nt:
                    load_x(j + 2)
                hj = emit_h_phase(j)
                if j == 0:
                    for hi, (ho, hs) in enumerate(H_CHUNKS):
                        real = min(H - ho, hs)
                        for ci, (co, _) in enumerate(C_CHUNKS):
                            t2 = consts.tile([hs, 128], f32r, tag=f"w2h{hi}c{ci}")
                            if real < hs:
                                nc.vector.memset(t2[:].bitcast(f32), 0.0)
                            nc.gpsimd.dma_start(
                                out=t2[:real, :],
                                in_=w2T_d[ho : ho + real, co : co + 128],
                            )
                            w2_sb[(hi, ci)] = t2
                if pending is not None:
                    emit_y_phase(*pending)
                pending = (j, *hj)
            emit_y_phase(*pending)

    nc.compile()
    return nc


def _get_program(cap: int):
    if cap not in _PROGRAM_CACHE:
        _PROGRAM_CACHE[cap] = _build_program(cap)
    return _PROGRAM_CACHE[cap]


def kernel(x, router_w, w1, w2, w3, _trace=False):
    B, T, _ = x.shape
    n = B * T
    flat = np.ascontiguousarray(x.reshape(n, C), dtype=np.float32)
    i0, i1, g0, g1 = _route(flat, np.asarray(router_w, dtype=np.float32))

    # Dispatch: for each expert, the token rows routed to it (slot0 then slot1).
    pos = np.empty((2, n), dtype=np.int64)  # row of each (slot, token) in Y
    in_maps = []
    counts = [
        (np.nonzero(i0 == e)[0], np.nonzero(i1 == e)[0]) for e in range(E)
    ]
    cap = max(len(s0) + len(s1) for s0, s1 in counts)
    cap = max(((cap + 127) // 128) * 128, 256)

    w1 = np.asarray(w1, dtype=np.float32)
    w2 = np.asarray(w2, dtype=np.float32)
    w3 = np.asarray(w3, dtype=np.float32)
    for e in range(E):
        s0, s1 = counts[e]
        cnt = len(s0) + len(s1)
        base = e * cap
        pos[0, s0] = base + np.arange(len(s0))
        pos[1, s1] = base + len(s0) + np.arange(len(s1))

        xT = np.zeros((C, cap), dtype=np.float32)
        xT[:, : len(s0)] = flat[s0].T
        xT[:, len(s0) : cnt] = flat[s1].T
        g = np.zeros((1, cap), dtype=np.float32)
        g[0, : len(s0)] = g0[s0]
        g[0, len(s0) : cnt] = g1[s1]
        in_maps.append(
            {
                "xT": xT,
                "g": g,
                "w1T": np.ascontiguousarray(w1[e].T),
                "w3T": np.ascontiguousarray(w3[e].T),
                "w2T": np.ascontiguousarray(w2[e].T),
            }
        )

    nc = _get_program(cap)
    if _trace:
        res = run_bass_kernel_spmd(nc, in_maps, list(range(E)), trace=True)
    else:
        # The NTFF trace path needs an antenv.axon_hooks shim this module
        # doesn't install; make sure an ambient BASS_TRACE can't enable it.
        prev = os.environ.get("BASS_NEVER_TRACE")
        os.environ["BASS_NEVER_TRACE"] = "1"
        try:
            res = run_bass_kernel_spmd(nc, in_maps, list(range(E)), trace=False)
        finally:
            if prev is None:
                os.environ.pop("BASS_NEVER_TRACE", None)
            else:
                os.environ["BASS_NEVER_TRACE"] = prev

    Y = np.empty((E * cap, C), dtype=np.float32)
    for e in range(E):
        Y[e * cap : (e + 1) * cap] = res.results[e]["yT"].T.astype(np.float32)
    out = Y[pos[0]] + Y[pos[1]]
    if _trace:
        kernel.last_results = res
    return out.reshape(B, T, C)


# revision 13
# speedup vs baseline: 1.2370x; 1.0704x over previous
"""MoE layer (8 experts, top-2 routing, SwiGLU) on 8 Trainium2 NeuronCores.

Strategy (expert-parallel + overflow spill, f32r):
  - Host: router in numpy; expert-sorted dispatch. Each core processes QA
    "primary" columns of its own expert plus one R-wide "helper" block
    holding overflow tokens of (possibly) another expert — the helper's
    weight set is per-core input data, so the SPMD program structure stays
    identical while per-core capacity drops from max_e(cnt_e) to QA+R.
  - Device (SPMD): y = (silu(x @ w1.T) * (x @ w3.T)) @ w2.T, gate applied
    on the y side via a broadcast row. f32r matmuls (1 cycle/row for
    moving >= 256; measured faster per-instruction than bf16 here).
  - Host: un-permute and add the two expert contributions per token.

B, T, C, E, H = 8, 2048, 256, 8, 682; N = B*T = 16384 tokens, top-2.
"""

import os

import ml_dtypes
import numpy as np

import concourse.bass as bass
import concourse.tile as tile
from concourse import bacc, mybir
from concourse.bass_utils import run_bass_kernel_spmd

E = 8
TOP_K = 2
C = 256
H = 682
HP = 768  # H zero-padded to a multiple of 128 (zero weights are exact)
NTILE = 512  # moving-dim tile (fp32 PSUM bank width)
H_CHUNKS = [(i * 128, 128) for i in range(HP // 128)]
C_CHUNKS = [(i * 128, 128) for i in range(C // 128)]
N_WARMUP_MM = 8  # fp32 dummy matmuls covering the startup DMA window
# (~3.6us: enough sustained PE activity to flip the HAM to full clock
# just as the first x tile lands)

BF16 = ml_dtypes.bfloat16

_PROGRAM_CACHE: dict[tuple, object] = {}


def _route(flat: np.ndarray, router_w: np.ndarray):
    """Replicates the reference router: softmax over experts, top-2, renorm."""
    logits = flat @ router_w.T  # [N, E]
    logits -= logits.max(axis=-1, keepdims=True)
    probs = np.exp(logits)
    probs /= probs.sum(axis=-1, keepdims=True)

    n = flat.shape[0]
    ar = np.arange(n)
    i0 = probs.argmax(axis=-1)
    p0 = probs[ar, i0]
    masked = probs.copy()
    masked[ar, i0] = -np.inf
    i1 = masked.argmax(axis=-1)
    p1 = probs[ar, i1]
    denom = p0 + p1 + 1e-9
    return i0, i1, (p0 / denom).astype(np.float32), (p1 / denom).astype(np.float32)


def _n_tiles(cap: int):
    """n-tile sizes: a small (256..512) head tile so the first matmul only
    waits on a short DMA, then 512-wide tiles (>=256, the f32r fast-path
    minimum)."""
    rem = cap % NTILE
    if rem == 0:
        sizes = [NTILE] * (cap // NTILE)
    elif rem >= 256:
        sizes = [rem] + [NTILE] * (cap // NTILE)
    else:
        a = (rem + NTILE) // 2
        a = ((a + 15) // 16) * 16
        sizes = [a, rem + NTILE - a] + [NTILE] * (cap // NTILE - 1)
    assert sum(sizes) == cap
    return sizes


def _plan(counts):
    """Pick (QA, R) minimizing per-core columns QA+R such that every
    expert's overflow beyond QA packs into at most 8 R-wide helper blocks
    (one per core). Always feasible: QA = round128(max) gives overflow 0."""
    cnt = np.asarray(counts)
    hi = ((int(cnt.max()) + 127) // 128) * 128
    best = (hi + 256, hi, 256)
    for QA in range(3072, hi + 1, 128):
        for R in (256, 384, 512):
            ov = np.maximum(cnt - QA, 0)
            if int(np.ceil(ov / R).sum()) <= E and QA + R < best[0]:
                best = (QA + R, QA, R)
    _, QA, R = best
    # helper block assignment: (expert, slot_offset, length) per block
    blocks = []
    for e in range(E):
        s = QA
        while s < cnt[e]:
            ln = min(R, cnt[e] - s)
            blocks.append((e, s, int(ln)))
            s += ln
    blocks += [None] * (E - len(blocks))
    return QA, R, blocks


def _build_program(qa_sizes: tuple, r: int):
    f32 = mybir.dt.float32
    f32r = mybir.dt.float32r
    bf16 = mybir.dt.bfloat16
    sizes = list(qa_sizes) + [r]
    wset_of = [0] * len(qa_sizes) + [1]
    ntiles, off = [], 0
    for s in sizes:
        ntiles.append((off, s))
        off += s
    cap = off
    nt = len(ntiles)

    nc = bacc.Bacc(
        "TRN2",
        target_bir_lowering=False,
        debug=False,
        enable_asserts=False,
        num_devices=E,
    )
    xT_d = nc.dram_tensor("xT", [C, cap], f32r, kind="ExternalInput").ap()
    g_d = nc.dram_tensor("g", [1, cap], bf16, kind="ExternalInput").ap()
    w_d = {}
    for s, sfx in ((0, ""), (1, "B")):
        w_d[(s, 1)] = nc.dram_tensor(f"w1T{sfx}", [C, H], f32r, kind="ExternalInput").ap()
        w_d[(s, 3)] = nc.dram_tensor(f"w3T{sfx}", [C, H], f32r, kind="ExternalInput").ap()
        w_d[(s, 2)] = nc.dram_tensor(f"w2T{sfx}", [H, C], f32r, kind="ExternalInput").ap()
    yT_d = nc.dram_tensor("yT", [C, cap], bf16, kind="ExternalOutput").ap()

    with tile.TileContext(nc) as tc:
        with (
            tc.tile_pool(name="consts", bufs=1) as consts,
            tc.tile_pool(name="xin", bufs=3) as xin,
            tc.tile_pool(name="gbp", bufs=3) as gbpool,
            tc.tile_pool(name="hbuf", bufs=3) as hbuf,
            tc.tile_pool(name="act", bufs=4) as actp,
            tc.tile_pool(name="yout", bufs=4) as yout,
            tc.tile_pool(name="ps_h", bufs=2, space="PSUM") as ps_h,
            tc.tile_pool(name="ps_y", bufs=3, space="PSUM") as ps_y,
            tc.tile_pool(name="ps_w", bufs=1, space="PSUM") as ps_w,
        ):
            # PE warm-up: dummy matmuls on zeroed SBUF keep the HAM busy
            # (and warm) while the first input DMAs are in flight.
            wz_l = consts.tile([128, 64], f32, tag="wz_l")
            nc.vector.memset(wz_l[:], 0.0)
            for _ in range(N_WARMUP_MM):
                wp = ps_w.tile([64, 64], f32, tag="warm")
                nc.tensor.matmul(wp[:], wz_l[:, :64], wz_l[:], start=True, stop=True)
            wp_last = wp

            x_tiles: dict[int, list] = {}

            def load_x(j):
                no, nsz = ntiles[j]
                ts = []
                for ci, (co, _) in enumerate(C_CHUNKS):
                    xt = xin.tile([128, nsz], f32r, tag=f"x{ci}")
                    nc.sync.dma_start(
                        out=xt[:], in_=xT_d[co : co + 128, no : no + nsz]
                    )
                    ts.append(xt)
                x_tiles[j] = ts

            # Critical-path first: the opening matmul needs x(j0,c0) + the
            # first w1 chunk. Weights land as per-[128,128] tiles so the
            # first matmul waits on 64KB, not a 350KB block. w2 and the
            # helper weight set ride the gpsimd queue later.
            w1_sb = {0: {}, 1: {}}
            w3_sb = {0: {}, 1: {}}
            w2_sb = {0: {}, 1: {}}
            load_x(0)
            # head tiles: H columns [0,256) per (w, C-chunk) — the first
            # matmuls wait only on these; 1KB DMA rows
            for w_sb, wn in ((w1_sb, 1), (w3_sb, 3)):
                for ci, (co, _) in enumerate(C_CHUNKS):
                    tA = consts.tile([128, 256], f32r, tag=f"w{wn}c{ci}A")
                    nc.sync.dma_start(
                        out=tA[:], in_=w_d[(0, wn)][co : co + 128, 0:256]
                    )
                    for hi in (0, 1):
                        w_sb[0][(ci, hi)] = tA[:, hi * 128 : hi * 128 + 128]
            load_x(1)
            # tail tiles: H columns [256,768) (real 256:682 + zero pad);
            # 1.7KB DMA rows keep the descriptor rate efficient
            for w_sb, wn in ((w1_sb, 1), (w3_sb, 3)):
                for ci, (co, _) in enumerate(C_CHUNKS):
                    tB = consts.tile([128, 512], f32r, tag=f"w{wn}c{ci}B")
                    nc.vector.memset(tB[:, H - 256 :].bitcast(f32), 0.0)
                    nc.sync.dma_start(
                        out=tB[:, : H - 256], in_=w_d[(0, wn)][co : co + 128, 256:H]
                    )
                    for hi in (2, 3, 4, 5):
                        w_sb[0][(ci, hi)] = tB[:, (hi - 2) * 128 : (hi - 1) * 128]
            load_x(2)

            def defer(t, src_t=None):
                # 1-column garbage write sourced from an earlier pipeline
                # point: the following DMA (WAW on the tile) waits for it
                # instead of launching at t0.
                s_ = wp_last[:, 0:1] if src_t is None else src_t[:64, 0:1]
                nc.vector.tensor_copy(t[:64, 0:1], s_)

            def load_w2(s, src_t=None):
                for hi, (ho, hs) in enumerate(H_CHUNKS):
                    real = min(H - ho, hs)
                    for ci, (co, _) in enumerate(C_CHUNKS):
                        t2 = consts.tile([hs, 128], f32r, tag=f"w2h{hi}c{ci}s{s}")
                        if real < hs:
                            nc.vector.memset(t2[:].bitcast(f32), 0.0)
                        defer(t2, src_t)
                        nc.gpsimd.dma_start(
                            out=t2[:real, :],
                            in_=w_d[(s, 2)][ho : ho + real, co : co + 128],
                        )
                        w2_sb[s][(hi, ci)] = t2

            def load_w13_b(his, src_t=None):
                for hi in his:
                    ho, hs = H_CHUNKS[hi]
                    he = min(ho + hs, H)
                    for w_sb, wn in ((w1_sb, 1), (w3_sb, 3)):
                        for ci, (co, _) in enumerate(C_CHUNKS):
                            t = consts.tile([128, hs], f32r, tag=f"w{wn}c{ci}h{hi}B")
                            if he < ho + hs:
                                nc.vector.memset(t[:, he - ho :].bitcast(f32), 0.0)
                            defer(t, src_t)
                            nc.gpsimd.dma_start(
                                out=t[:, : he - ho],
                                in_=w_d[(1, wn)][co : co + 128, ho:he],
                            )
                            w_sb[1][(ci, hi)] = t[:, :]

            def emit_h_phase(j):
                """h = silu(x@w1T) * (x@w3T) for n-tile j; returns SBUF tiles."""
                no, nsz = ntiles[j]
                s = wset_of[j]
                x_sb = x_tiles.pop(j)
                h_tiles = []
                for hi, (ho, hs) in enumerate(H_CHUNKS):
                    h1p = ps_h.tile([hs, nsz], f32, tag="h1")
                    h3p = ps_h.tile([hs, nsz], f32, tag="h3")
                    for ci in range(len(C_CHUNKS)):
                        first = ci == 0
                        last = ci == len(C_CHUNKS) - 1
                        nc.tensor.matmul(
                            h1p[:], w1_sb[s][(ci, hi)], x_sb[ci][:],
                            start=first, stop=last,
                        )
                        nc.tensor.matmul(
                            h3p[:], w3_sb[s][(ci, hi)], x_sb[ci][:],
                            start=first, stop=last,
                        )
                    a_sb = actp.tile([hs, nsz], f32r, tag="a")
                    nc.scalar.activation(
                        a_sb[:], h1p[:], mybir.ActivationFunctionType.Silu
                    )
                    h_sb = hbuf.tile([hs, nsz], f32r, tag=f"h{hi}")
                    nc.vector.tensor_mul(h_sb[:], a_sb[:], h3p[:])
                    h_tiles.append(h_sb)
                # gate row broadcast to 128 partitions via stride-0 DMA
                gb_sb = gbpool.tile([128, nsz], bf16, tag="gb")
                g_slice = g_d[0:1, no : no + nsz]
                g_bcast = bass.AP(
                    tensor=g_slice.tensor,
                    offset=g_slice.offset,
                    ap=[[0, 128], list(g_slice.ap[-1])],
                )
                nc.gpsimd.dma_start(out=gb_sb[:], in_=g_bcast)
                return h_tiles, gb_sb

            def emit_y_phase(j, h_tiles, gb_sb):
                no, nsz = ntiles[j]
                s = wset_of[j]
                for ci, (co, _) in enumerate(C_CHUNKS):
                    yp = ps_y.tile([128, nsz], f32, tag="y")
                    for hi in range(len(H_CHUNKS)):
                        nc.tensor.matmul(
                            yp[:], w2_sb[s][(hi, ci)][:], h_tiles[hi][:],
                            start=hi == 0, stop=hi == len(H_CHUNKS) - 1,
                        )
                    y_sb = yout.tile([128, nsz], bf16, tag="yo")
                    nc.vector.tensor_mul(y_sb[:], yp[:], gb_sb[:])
                    nc.sync.dma_start(
                        out=yT_d[co : co + 128, no : no + nsz], in_=y_sb[:]
                    )

            # Software pipeline: y-phase of tile j is emitted after the
            # h-phase of tile j+1, so the PE never waits on the silu->mul
            # chain at the h->y boundary.
            pending = None
            for j in range(nt):
                if 2 < j + 2 < nt:
                    load_x(j + 2)
                hj = emit_h_phase(j)
                if j == 0:
                    load_w2(0)
                elif j == 2:
                    load_w13_b((0, 1, 2), pending[1][0])
                elif j == 3:
                    load_w13_b((3, 4, 5), pending[1][0])
                elif j == 4:
                    load_w2(1, pending[1][0])
                if pending is not None:
                    emit_y_phase(*pending)
                pending = (j, *hj)
            emit_y_phase(*pending)

    nc.compile()
    return nc


def _get_program(qa_sizes, r):
    key = (tuple(qa_sizes), r)
    if key not in _PROGRAM_CACHE:
        _PROGRAM_CACHE[key] = _build_program(tuple(qa_sizes), r)
    return _PROGRAM_CACHE[key]


def kernel(x, router_w, w1, w2, w3, _trace=False):
    B, T, _ = x.shape
    n = B * T
    flat = np.ascontiguousarray(x.reshape(n, C), dtype=np.float32)
    i0, i1, g0, g1 = _route(flat, np.asarray(router_w, dtype=np.float32))

    # Dispatch: for each expert, the token rows routed to it (slot0 then
    # slot1), as one ordered slot list per expert.
    rows_e, gates_e = [], []
    for e in range(E):
        s0 = np.nonzero(i0 == e)[0]
        s1 = np.nonzero(i1 == e)[0]
        rows_e.append(np.concatenate([s0, s1]))
        gates_e.append(np.concatenate([g0[s0], g1[s1]]).astype(np.float32))
    counts = [len(r_) for r_ in rows_e]
    QA, R, blocks = _plan(counts)
    CT = QA + R

    w1 = np.asarray(w1, dtype=np.float32)
    w2 = np.asarray(w2, dtype=np.float32)
    w3 = np.asarray(w3, dtype=np.float32)
    w1T = [np.ascontiguousarray(w1[e].T) for e in range(E)]
    w3T = [np.ascontiguousarray(w3[e].T) for e in range(E)]
    w2T = [np.ascontiguousarray(w2[e].T) for e in range(E)]
    zw13 = np.zeros((C, H), dtype=np.float32)
    zw2 = np.zeros((H, C), dtype=np.float32)

    # Y row of every (expert, slot) position
    yrow_e = [np.empty(counts[e], dtype=np.int64) for e in range(E)]
    in_maps = []
    for c in range(E):
        xT = np.zeros((C, CT), dtype=np.float32)
        g = np.zeros((1, CT), dtype=np.float32)
        # primary: expert c, slots [0, min(cnt, QA))
        np_ = min(counts[c], QA)
        xT[:, :np_] = flat[rows_e[c][:np_]].T
        g[0, :np_] = gates_e[c][:np_]
        yrow_e[c][:np_] = c * CT + np.arange(np_)
        # helper block
        blk = blocks[c]
        if blk is not None:
            e, off, ln = blk
            xT[:, QA : QA + ln] = flat[rows_e[e][off : off + ln]].T
            g[0, QA : QA + ln] = gates_e[e][off : off + ln]
            yrow_e[e][off : off + ln] = c * CT + QA + np.arange(ln)
            w1b, w3b, w2b = w1T[e], w3T[e], w2T[e]
        else:
            w1b, w3b, w2b = zw13, zw13, zw2
        in_maps.append(
            {
                "xT": xT,
                "g": g.astype(BF16),
                "w1T": w1T[c],
                "w3T": w3T[c],
                "w2T": w2T[c],
                "w1TB": w1b,
                "w3TB": w3b,
                "w2TB": w2b,
            }
        )

    pos = np.empty((2, n), dtype=np.int64)
    for e in range(E):
        r_ = rows_e[e]
        n0 = len(np.nonzero(i0 == e)[0])
        pos[0, r_[:n0]] = yrow_e[e][:n0]
        pos[1, r_[n0:]] = yrow_e[e][n0:]

    nc = _get_program(_n_tiles(QA), R)
    if _trace:
        res = run_bass_kernel_spmd(nc, in_maps, list(range(E)), trace=True)
    else:
        prev = os.environ.get("BASS_NEVER_TRACE")
        os.environ["BASS_NEVER_TRACE"] = "1"
        try:
            res = run_bass_kernel_spmd(nc, in_maps, list(range(E)), trace=False)
        finally:
            if prev is None:
                os.environ.pop("BASS_NEVER_TRACE", None)
            else:
                os.environ["BASS_NEVER_TRACE"] = prev

    Y = np.empty((E * CT, C), dtype=np.float32)
    for c in range(E):
        Y[c * CT : (c + 1) * CT] = res.results[c]["yT"].T.astype(np.float32)
    out = Y[pos[0]] + Y[pos[1]]
    if _trace:
        kernel.last_results = res
    return out.reshape(B, T, C)


# revision 14
# speedup vs baseline: 1.2756x; 1.0312x over previous
"""MoE layer (8 experts, top-2 routing, SwiGLU) on 8 Trainium2 NeuronCores.

Strategy (expert-parallel + overflow spill, f32r):
  - Host: router in numpy; expert-sorted dispatch. Each core processes QA
    "primary" columns of its own expert plus one R-wide "helper" block
    holding overflow tokens of (possibly) another expert — the helper's
    weight set is per-core input data, so the SPMD program structure stays
    identical while per-core capacity drops from max_e(cnt_e) to QA+R.
  - Device (SPMD): y = (silu(x @ w1.T) * (x @ w3.T)) @ w2.T, gate applied
    on the y side via a broadcast row. f32r matmuls (1 cycle/row for
    moving >= 256; measured faster per-instruction than bf16 here).
  - Host: un-permute and add the two expert contributions per token.

B, T, C, E, H = 8, 2048, 256, 8, 682; N = B*T = 16384 tokens, top-2.
"""

import os

import ml_dtypes
import numpy as np

import concourse.bass as bass
import concourse.tile as tile
from concourse import bacc, mybir
from concourse.bass_utils import run_bass_kernel_spmd

E = 8
TOP_K = 2
C = 256
H = 682
HP = 768  # H zero-padded to a multiple of 128 (zero weights are exact)
NTILE = 512  # moving-dim tile (fp32 PSUM bank width)
H_CHUNKS = [(i * 128, 128) for i in range(HP // 128)]
C_CHUNKS = [(i * 128, 128) for i in range(C // 128)]
N_WARMUP_MM = 10  # fp32 dummy matmuls covering the startup DMA window
# (~3.6us: enough sustained PE activity to flip the HAM to full clock
# just as the first x tile lands)

BF16 = ml_dtypes.bfloat16

_PROGRAM_CACHE: dict[tuple, object] = {}


def _route(flat: np.ndarray, router_w: np.ndarray):
    """Replicates the reference router: softmax over experts, top-2, renorm."""
    logits = flat @ router_w.T  # [N, E]
    logits -= logits.max(axis=-1, keepdims=True)
    probs = np.exp(logits)
    probs /= probs.sum(axis=-1, keepdims=True)

    n = flat.shape[0]
    ar = np.arange(n)
    i0 = probs.argmax(axis=-1)
    p0 = probs[ar, i0]
    masked = probs.copy()
    masked[ar, i0] = -np.inf
    i1 = masked.argmax(axis=-1)
    p1 = probs[ar, i1]
    denom = p0 + p1 + 1e-9
    return i0, i1, (p0 / denom).astype(np.float32), (p1 / denom).astype(np.float32)


def _n_tiles(cap: int):
    """n-tile sizes: a small (256..512) head tile so the first matmul only
    waits on a short DMA, then 512-wide tiles (>=256, the f32r fast-path
    minimum)."""
    rem = cap % NTILE
    if rem == 0:
        sizes = [NTILE] * (cap // NTILE)
    elif rem >= 256:
        sizes = [rem] + [NTILE] * (cap // NTILE)
    else:
        a = (rem + NTILE) // 2
        a = ((a + 15) // 16) * 16
        sizes = [a, rem + NTILE - a] + [NTILE] * (cap // NTILE - 1)
    assert sum(sizes) == cap
    return sizes


def _plan(counts):
    """Pick (QA, R) minimizing per-core columns QA+R such that every
    expert's overflow beyond QA packs into at most 8 R-wide helper blocks
    (one per core). Always feasible: QA = round128(max) gives overflow 0."""
    cnt = np.asarray(counts)
    hi = ((int(cnt.max()) + 127) // 128) * 128
    best = (hi + 256, hi, 256)
    for QA in range(3072, hi + 1, 128):
        for R in (256, 384, 512):
            ov = np.maximum(cnt - QA, 0)
            if int(np.ceil(ov / R).sum()) <= E and QA + R < best[0]:
                best = (QA + R, QA, R)
    _, QA, R = best
    # helper block assignment: (expert, slot_offset, length) per block
    blocks = []
    for e in range(E):
        s = QA
        while s < cnt[e]:
            ln = min(R, cnt[e] - s)
            blocks.append((e, s, int(ln)))
            s += ln
    blocks += [None] * (E - len(blocks))
    return QA, R, blocks


def _build_program(qa_sizes: tuple, r: int):
    f32 = mybir.dt.float32
    f32r = mybir.dt.float32r
    bf16 = mybir.dt.bfloat16
    sizes = list(qa_sizes) + [r]
    wset_of = [0] * len(qa_sizes) + [1]
    ntiles, off = [], 0
    for s in sizes:
        ntiles.append((off, s))
        off += s
    cap = off
    nt = len(ntiles)

    nc = bacc.Bacc(
        "TRN2",
        target_bir_lowering=False,
        debug=False,
        enable_asserts=False,
        num_devices=E,
    )
    xT_d = nc.dram_tensor("xT", [C, cap], f32r, kind="ExternalInput").ap()
    g_d = nc.dram_tensor("g", [1, cap], bf16, kind="ExternalInput").ap()
    w_d = {}
    for s, sfx in ((0, ""), (1, "B")):
        w_d[(s, 1)] = nc.dram_tensor(f"w1T{sfx}", [C, H], f32r, kind="ExternalInput").ap()
        w_d[(s, 3)] = nc.dram_tensor(f"w3T{sfx}", [C, H], f32r, kind="ExternalInput").ap()
        w_d[(s, 2)] = nc.dram_tensor(f"w2T{sfx}", [H, C], f32r, kind="ExternalInput").ap()
    yT_d = nc.dram_tensor("yT", [C, cap], bf16, kind="ExternalOutput").ap()

    with tile.TileContext(nc) as tc:
        with (
            tc.tile_pool(name="consts", bufs=1) as consts,
            tc.tile_pool(name="xin", bufs=3) as xin,
            tc.tile_pool(name="gbp", bufs=3) as gbpool,
            tc.tile_pool(name="hbuf", bufs=3) as hbuf,
            tc.tile_pool(name="act", bufs=4) as actp,
            tc.tile_pool(name="yout", bufs=4) as yout,
            tc.tile_pool(name="ps_h", bufs=2, space="PSUM") as ps_h,
            tc.tile_pool(name="ps_y", bufs=3, space="PSUM") as ps_y,
            tc.tile_pool(name="ps_w", bufs=1, space="PSUM") as ps_w,
        ):
            # PE warm-up: dummy matmuls on zeroed SBUF keep the HAM busy
            # (and warm) while the first input DMAs are in flight.
            wz_l = consts.tile([128, 64], f32, tag="wz_l")
            nc.vector.memset(wz_l[:], 0.0)
            for _ in range(N_WARMUP_MM):
                wp = ps_w.tile([64, 64], f32, tag="warm")
                nc.tensor.matmul(wp[:], wz_l[:, :64], wz_l[:], start=True, stop=True)
            wp_last = wp

            x_tiles: dict[int, list] = {}

            def load_x(j):
                no, nsz = ntiles[j]
                ts = []
                for ci, (co, _) in enumerate(C_CHUNKS):
                    xt = xin.tile([128, nsz], f32r, tag=f"x{ci}")
                    nc.sync.dma_start(
                        out=xt[:], in_=xT_d[co : co + 128, no : no + nsz]
                    )
                    ts.append(xt)
                x_tiles[j] = ts

            # Critical-path first: the opening matmul needs x(j0,c0) + the
            # first w1 chunk. Weights land as per-[128,128] tiles so the
            # first matmul waits on 64KB, not a 350KB block. w2 and the
            # helper weight set ride the gpsimd queue later.
            w1_sb = {0: {}, 1: {}}
            w3_sb = {0: {}, 1: {}}
            w2_sb = {0: {}, 1: {}}
            load_x(0)
            # head tiles: H columns [0,256) per (w, C-chunk) — the first
            # matmuls wait only on these; 1KB DMA rows
            for w_sb, wn in ((w1_sb, 1), (w3_sb, 3)):
                for ci, (co, _) in enumerate(C_CHUNKS):
                    tA = consts.tile([128, 256], f32r, tag=f"w{wn}c{ci}A")
                    nc.sync.dma_start(
                        out=tA[:], in_=w_d[(0, wn)][co : co + 128, 0:256]
                    )
                    for hi in (0, 1):
                        w_sb[0][(ci, hi)] = tA[:, hi * 128 : hi * 128 + 128]
            load_x(1)
            # tail tiles: H columns [256,768) (real 256:682 + zero pad);
            # 1.7KB DMA rows keep the descriptor rate efficient
            for w_sb, wn in ((w1_sb, 1), (w3_sb, 3)):
                for ci, (co, _) in enumerate(C_CHUNKS):
                    tB = consts.tile([128, 512], f32r, tag=f"w{wn}c{ci}B")
                    nc.vector.memset(tB[:, H - 256 :].bitcast(f32), 0.0)
                    nc.sync.dma_start(
                        out=tB[:, : H - 256], in_=w_d[(0, wn)][co : co + 128, 256:H]
                    )
                    for hi in (2, 3, 4, 5):
                        w_sb[0][(ci, hi)] = tB[:, (hi - 2) * 128 : (hi - 1) * 128]
            load_x(2)

            def defer(t, src_t=None):
                # 1-column garbage write sourced from an earlier pipeline
                # point: the following DMA (WAW on the tile) waits for it
                # instead of launching at t0.
                s_ = wp_last[:, 0:1] if src_t is None else src_t[:64, 0:1]
                nc.vector.tensor_copy(t[:64, 0:1], s_)

            def load_w2(s, src_t=None):
                for hi, (ho, hs) in enumerate(H_CHUNKS):
                    real = min(H - ho, hs)
                    for ci, (co, _) in enumerate(C_CHUNKS):
                        t2 = consts.tile([hs, 128], f32r, tag=f"w2h{hi}c{ci}s{s}")
                        if real < hs:
                            nc.vector.memset(t2[:].bitcast(f32), 0.0)
                        defer(t2, src_t)
                        nc.gpsimd.dma_start(
                            out=t2[:real, :],
                            in_=w_d[(s, 2)][ho : ho + real, co : co + 128],
                        )
                        w2_sb[s][(hi, ci)] = t2

            def load_w13_b(his, src_t=None):
                for hi in his:
                    ho, hs = H_CHUNKS[hi]
                    he = min(ho + hs, H)
                    for w_sb, wn in ((w1_sb, 1), (w3_sb, 3)):
                        for ci, (co, _) in enumerate(C_CHUNKS):
                            t = consts.tile([128, hs], f32r, tag=f"w{wn}c{ci}h{hi}B")
                            if he < ho + hs:
                                nc.vector.memset(t[:, he - ho :].bitcast(f32), 0.0)
                            defer(t, src_t)
                            nc.gpsimd.dma_start(
                                out=t[:, : he - ho],
                                in_=w_d[(1, wn)][co : co + 128, ho:he],
                            )
                            w_sb[1][(ci, hi)] = t[:, :]

            def emit_h_phase(j):
                """h = silu(x@w1T) * (x@w3T) for n-tile j; returns SBUF tiles."""
                no, nsz = ntiles[j]
                s = wset_of[j]
                x_sb = x_tiles.pop(j)
                h_tiles = []
                for hi, (ho, hs) in enumerate(H_CHUNKS):
                    h1p = ps_h.tile([hs, nsz], f32, tag="h1")
                    h3p = ps_h.tile([hs, nsz], f32, tag="h3")
                    for ci in range(len(C_CHUNKS)):
                        first = ci == 0
                        last = ci == len(C_CHUNKS) - 1
                        nc.tensor.matmul(
                            h1p[:], w1_sb[s][(ci, hi)], x_sb[ci][:],
                            start=first, stop=last,
                        )
                        nc.tensor.matmul(
                            h3p[:], w3_sb[s][(ci, hi)], x_sb[ci][:],
                            start=first, stop=last,
                        )
                    a_sb = actp.tile([hs, nsz], f32r, tag="a")
                    nc.scalar.activation(
                        a_sb[:], h1p[:], mybir.ActivationFunctionType.Silu
                    )
                    h_sb = hbuf.tile([hs, nsz], f32r, tag=f"h{hi}")
                    nc.vector.tensor_mul(h_sb[:], a_sb[:], h3p[:])
                    h_tiles.append(h_sb)
                # gate row broadcast to 128 partitions via stride-0 DMA
                gb_sb = gbpool.tile([128, nsz], bf16, tag="gb")
                g_slice = g_d[0:1, no : no + nsz]
                g_bcast = bass.AP(
                    tensor=g_slice.tensor,
                    offset=g_slice.offset,
                    ap=[[0, 128], list(g_slice.ap[-1])],
                )
                nc.gpsimd.dma_start(out=gb_sb[:], in_=g_bcast)
                return h_tiles, gb_sb

            def emit_y_phase(j, h_tiles, gb_sb):
                no, nsz = ntiles[j]
                s = wset_of[j]
                for ci, (co, _) in enumerate(C_CHUNKS):
                    yp = ps_y.tile([128, nsz], f32, tag="y")
                    for hi in range(len(H_CHUNKS)):
                        nc.tensor.matmul(
                            yp[:], w2_sb[s][(hi, ci)][:], h_tiles[hi][:],
                            start=hi == 0, stop=hi == len(H_CHUNKS) - 1,
                        )
                    y_sb = yout.tile([128, nsz], bf16, tag="yo")
                    nc.vector.tensor_mul(y_sb[:], yp[:], gb_sb[:])
                    nc.sync.dma_start(
                        out=yT_d[co : co + 128, no : no + nsz], in_=y_sb[:]
                    )

            # Software pipeline: y-phase of tile j is emitted after the
            # h-phase of tile j+1, so the PE never waits on the silu->mul
            # chain at the h->y boundary.
            pending = None
            for j in range(nt):
                if 2 < j + 2 < nt:
                    load_x(j + 2)
                hj = emit_h_phase(j)
                if j == 0:
                    load_w2(0, hj[0][0])
                elif j == 2:
                    load_w13_b((0, 1, 2), pending[1][0])
                elif j == 3:
                    load_w13_b((3, 4, 5), pending[1][0])
                elif j == 4:
                    load_w2(1, pending[1][0])
                if pending is not None:
                    emit_y_phase(*pending)
                pending = (j, *hj)
            emit_y_phase(*pending)

    nc.compile()
    return nc


def _get_program(qa_sizes, r):
    key = (tuple(qa_sizes), r)
    if key not in _PROGRAM_CACHE:
        _PROGRAM_CACHE[key] = _build_program(tuple(qa_sizes), r)
    return _PROGRAM_CACHE[key]


def kernel(x, router_w, w1, w2, w3, _trace=False):
    B, T, _ = x.shape
    n = B * T
    flat = np.ascontiguousarray(x.reshape(n, C), dtype=np.float32)
    i0, i1, g0, g1 = _route(flat, np.asarray(router_w, dtype=np.float32))

    # Dispatch: for each expert, the token rows routed to it (slot0 then
    # slot1), as one ordered slot list per expert.
    rows_e, gates_e = [], []
    for e in range(E):
        s0 = np.nonzero(i0 == e)[0]
        s1 = np.nonzero(i1 == e)[0]
        rows_e.append(np.concatenate([s0, s1]))
        gates_e.append(np.concatenate([g0[s0], g1[s1]]).astype(np.float32))
    counts = [len(r_) for r_ in rows_e]
    QA, R, blocks = _plan(counts)
    CT = QA + R

    w1 = np.asarray(w1, dtype=np.float32)
    w2 = np.asarray(w2, dtype=np.float32)
    w3 = np.asarray(w3, dtype=np.float32)
    w1T = [np.ascontiguousarray(w1[e].T) for e in range(E)]
    w3T = [np.ascontiguousarray(w3[e].T) for e in range(E)]
    w2T = [np.ascontiguousarray(w2[e].T) for e in range(E)]
    zw13 = np.zeros((C, H), dtype=np.float32)
    zw2 = np.zeros((H, C), dtype=np.float32)

    # Y row of every (expert, slot) position
    yrow_e = [np.empty(counts[e], dtype=np.int64) for e in range(E)]
    in_maps = []
    for c in range(E):
        xT = np.zeros((C, CT), dtype=np.float32)
        g = np.zeros((1, CT), dtype=np.float32)
        # primary: expert c, slots [0, min(cnt, QA))
        np_ = min(counts[c], QA)
        xT[:, :np_] = flat[rows_e[c][:np_]].T
        g[0, :np_] = gates_e[c][:np_]
        yrow_e[c][:np_] = c * CT + np.arange(np_)
        # helper block
        blk = blocks[c]
        if blk is not None:
            e, off, ln = blk
            xT[:, QA : QA + ln] = flat[rows_e[e][off : off + ln]].T
            g[0, QA : QA + ln] = gates_e[e][off : off + ln]
            yrow_e[e][off : off + ln] = c * CT + QA + np.arange(ln)
            w1b, w3b, w2b = w1T[e], w3T[e], w2T[e]
        else:
            w1b, w3b, w2b = zw13, zw13, zw2
        in_maps.append(
            {
                "xT": xT,
                "g": g.astype(BF16),
                "w1T": w1T[c],
                "w3T": w3T[c],
                "w2T": w2T[c],
                "w1TB": w1b,
                "w3TB": w3b,
                "w2TB": w2b,
            }
        )

    pos = np.empty((2, n), dtype=np.int64)
    for e in range(E):
        r_ = rows_e[e]
        n0 = len(np.nonzero(i0 == e)[0])
        pos[0, r_[:n0]] = yrow_e[e][:n0]
        pos[1, r_[n0:]] = yrow_e[e][n0:]

    nc = _get_program(_n_tiles(QA), R)
    if _trace:
        res = run_bass_kernel_spmd(nc, in_maps, list(range(E)), trace=True)
    else:
        prev = os.environ.get("BASS_NEVER_TRACE")
        os.environ["BASS_NEVER_TRACE"] = "1"
        try:
            res = run_bass_kernel_spmd(nc, in_maps, list(range(E)), trace=False)
        finally:
            if prev is None:
                os.environ.pop("BASS_NEVER_TRACE", None)
            else:
                os.environ["BASS_NEVER_TRACE"] = prev

    Y = np.empty((E * CT, C), dtype=np.float32)
    for c in range(E):
        Y[c * CT : (c + 1) * CT] = res.results[c]["yT"].T.astype(np.float32)
    out = Y[pos[0]] + Y[pos[1]]
    if _trace:
        kernel.last_results = res
    return out.reshape(B, T, C)
